# revision 3
# baseline (speedup 1.0000x reference)
"""MoE expert-group kernel for Trainium2 (8 NeuronCores).

Problem: T=2048 tokens, E=8 experts, D=1024, I=2048.
  out[t] = silu(x[t] @ w_gate[e]) * (x[t] @ w_up[e]) @ w_down[e],  e = expert_indices[t]

Strategy: expert parallelism. Host-side (numpy) routing gathers tokens by
expert (this is the "all-to-all"); core e runs expert e's dense
gate/up/silu/down pipeline; host scatters rows back.

On-chip formulation is fully transposed so no transposes are ever needed:
  gateT = Wg^T @ X^T        (stationary = 128x128 Wg block, moving = xT [128, C])
  hidT  = silu(gateT)*upT   (ACT + DVE, written bf16)
  outT  = Wd^T @ hidT       (stationary = 128x128 Wd block, moving = hT [128, C])
All matmul moving operands are [128, C] (C = per-expert token capacity), all
stationary operands are dense 128x128 weight blocks. Inputs are cast to bf16
on the host (halves weight DMA; PE runs at full bf16 rate); accumulation is
fp32 in PSUM and the output is fp32.
"""

import sys

import numpy as np

try:
    import concourse  # noqa: F401
except ImportError:  # grading env fallback
    sys.path.insert(0, "/opt/trn_rl_repo")

import ml_dtypes

T, E, D, I = 2048, 8, 1024, 2048
ND = D // 128  # 8 contraction tiles for gate/up
NI = I // 128  # 16 contraction tiles for down

_PROGRAM_CACHE = {}


def _build_program(C):
    """Build + compile the per-core Bass program for token capacity C."""
    import concourse.bass as bass  # noqa: F401
    import concourse.mybir as mybir
    import concourse.tile as tile
    from concourse import bacc

    BF = mybir.dt.bfloat16
    F32 = mybir.dt.float32

    nc = bacc.Bacc("TRN2", target_bir_lowering=False, debug=False, num_devices=E)
    xT_d = nc.dram_tensor("xT", [D, C], BF, kind="ExternalInput").ap()
    wg_d = nc.dram_tensor("wg", [D, I], BF, kind="ExternalInput").ap()
    wu_d = nc.dram_tensor("wu", [D, I], BF, kind="ExternalInput").ap()
    wd_d = nc.dram_tensor("wd", [I, D], BF, kind="ExternalInput").ap()
    outT_d = nc.dram_tensor("outT", [D, C], F32, kind="ExternalOutput").ap()

    # PSUM bank holds 2KB/partition = 512 fp32: split the moving dim if needed.
    n_chunks = -(-C // 512)
    chunks = [(n * 512, min(512, C - n * 512)) for n in range(n_chunks)]

    with tile.TileContext(nc) as tc:
        with (
            tc.tile_pool(name="xp", bufs=1) as xp,
            tc.tile_pool(name="wp", bufs=1) as wp,
            tc.tile_pool(name="hp", bufs=1) as hp,
            tc.tile_pool(name="sp", bufs=3) as sp,
            tc.tile_pool(name="op", bufs=3) as op,
            tc.tile_pool(name="pg", bufs=2, space="PSUM") as pg,
            tc.tile_pool(name="pu", bufs=2, space="PSUM") as pu,
            tc.tile_pool(name="po", bufs=2, space="PSUM") as po,
        ):
            # Resident inputs: x first (small, unblocks phase 1 fastest),
            # then gate/up weights, then down weights (needed only in phase 2).
            xT = []
            for d in range(ND):
                t = xp.tile([128, C], BF, tag=f"x{d}", name=f"xT{d}")
                nc.sync.dma_start(t[:], xT_d[d * 128 : (d + 1) * 128, :])
                xT.append(t)
            wg = []
            wu = []
            for d in range(ND):
                t = wp.tile([128, I], BF, tag=f"wg{d}", name=f"wg{d}")
                nc.sync.dma_start(t[:], wg_d[d * 128 : (d + 1) * 128, :])
                wg.append(t)
                t = wp.tile([128, I], BF, tag=f"wu{d}", name=f"wu{d}")
                nc.sync.dma_start(t[:], wu_d[d * 128 : (d + 1) * 128, :])
                wu.append(t)
            wd = []
            for i in range(NI):
                t = wp.tile([128, D], BF, tag=f"wd{i}", name=f"wd{i}")
                nc.sync.dma_start(t[:], wd_d[i * 128 : (i + 1) * 128, :])
                wd.append(t)

            # Phase 1: hidT[i] = silu(Wg^T x^T) * (Wu^T x^T), one 128-row
            # strip of the intermediate dim per iteration.
            hT = []
            for i in range(NI):
                isl = bass.ds(i * 128, 128)
                h_t = hp.tile([128, C], BF, tag=f"h{i}", name=f"hT{i}")
                for c0, cn in chunks:
                    csl = bass.ds(c0, cn)
                    g_ps = pg.tile([128, cn], F32, tag="g", name="g_ps")
                    u_ps = pu.tile([128, cn], F32, tag="u", name="u_ps")
                    for d in range(ND):
                        nc.tensor.matmul(
                            g_ps[:],
                            wg[d][:, isl],
                            xT[d][:, csl],
                            start=(d == 0),
                            stop=(d == ND - 1),
                        )
                    for d in range(ND):
                        nc.tensor.matmul(
                            u_ps[:],
                            wu[d][:, isl],
                            xT[d][:, csl],
                            start=(d == 0),
                            stop=(d == ND - 1),
                        )
                    # silu(g) = g * sigmoid(g); each DVE mul reads at most
                    # one PSUM operand (DVE has a single PSUM read port).
                    s_sb = sp.tile([128, cn], F32, tag="s", name="s_sb")
                    nc.scalar.activation(
                        s_sb[:], g_ps[:], mybir.ActivationFunctionType.Sigmoid
                    )
                    gs_sb = sp.tile([128, cn], F32, tag="gs", name="gs_sb")
                    nc.vector.tensor_mul(gs_sb[:], s_sb[:], g_ps[:])
                    nc.vector.tensor_mul(h_t[:, csl], gs_sb[:], u_ps[:])
                hT.append(h_t)

            # Phase 2: outT[dstrip] = Wd^T @ hidT, accumulated over all 16
            # intermediate strips.
            for d in range(ND):
                dsl = bass.ds(d * 128, 128)
                for c0, cn in chunks:
                    csl = bass.ds(c0, cn)
                    o_ps = po.tile([128, cn], F32, tag="o", name="o_ps")
                    for i in range(NI):
                        nc.tensor.matmul(
                            o_ps[:],
                            wd[i][:, dsl],
                            hT[i][:, csl],
                            start=(i == 0),
                            stop=(i == NI - 1),
                        )
                    o_sb = op.tile([128, cn], F32, tag="ob", name="o_sb")
                    nc.vector.tensor_copy(o_sb[:], o_ps[:])
                    nc.sync.dma_start(outT_d[dsl, csl], o_sb[:])

    nc.compile()
    return nc


def _get_program(C):
    if C not in _PROGRAM_CACHE:
        _PROGRAM_CACHE[C] = _build_program(C)
    return _PROGRAM_CACHE[C]


def _run(nc, in_maps, trace=False):
    from concourse.bass_utils import run_bass_kernel_spmd

    return run_bass_kernel_spmd(nc, in_maps, core_ids=list(range(E)), trace=trace)


def kernel(x, expert_indices, w_gate, w_up, w_down, _trace=False, _results=None):
    x = np.asarray(x)
    idx = np.asarray(expert_indices).astype(np.int64)
    counts = np.bincount(idx, minlength=E)
    C = int(max(128, -(-counts.max() // 32) * 32))

    nc = _get_program(C)

    order = np.argsort(idx, kind="stable")
    starts = np.zeros(E + 1, dtype=np.int64)
    np.cumsum(counts, out=starts[1:])

    bf16 = ml_dtypes.bfloat16
    in_maps = []
    for e in range(E):
        toks = order[starts[e] : starts[e + 1]]
        xTg = np.zeros((D, C), dtype=bf16)
        xTg[:, : len(toks)] = x[toks].astype(bf16).T
        in_maps.append(
            {
                "xT": xTg,
                "wg": np.ascontiguousarray(w_gate[e]).astype(bf16),
                "wu": np.ascontiguousarray(w_up[e]).astype(bf16),
                "wd": np.ascontiguousarray(w_down[e]).astype(bf16),
            }
        )

    res = _run(nc, in_maps, trace=_trace)
    if _results is not None:
        _results.append(res)

    out = np.zeros((T, D), dtype=np.float32)
    for e in range(E):
        toks = order[starts[e] : starts[e + 1]]
        outT = res.results[e]["outT"]  # [D, C] fp32
        out[toks] = outT[:, : len(toks)].T
    return out


# revision 4
# speedup vs baseline: 1.0104x; 1.0104x over previous
"""MoE expert-group kernel for Trainium2 (8 NeuronCores).

Problem: T=2048 tokens, E=8 experts, D=1024, I=2048.
  out[t] = silu(x[t] @ w_gate[e]) * (x[t] @ w_up[e]) @ w_down[e],  e = expert_indices[t]

Strategy: expert parallelism. Host-side (numpy) routing gathers tokens by
expert (this is the "all-to-all"); core e runs expert e's dense
gate/up/silu/down pipeline; host scatters rows back.

On-chip formulation is fully transposed so no transposes are ever needed:
  gateT = Wg^T @ X^T        (stationary = 128x128 Wg block, moving = xT [128, C])
  hidT  = silu(gateT)*upT   (ACT sigmoid + DVE muls, written bf16)
  outT  = Wd^T @ hidT       (stationary = 128x128 Wd block, moving = hT [128, C])

All inputs are cast to bf16 on the host (halves weight DMA, PE runs at full
bf16 rate); accumulation is fp32 in PSUM and the output is fp32.

Weights for gate/up are repacked host-side into I-major 256KB blocks
([128, D] per 128-wide intermediate slice) so each phase-1 step depends only
on its own block, letting the PE chase the DMA stream instead of waiting for
a whole 4MB projection. Gate blocks stream on the Sync HWDGE ring, up blocks
on the Scalar ring, and the down-projection blocks queue behind them (FIFO
per ring) so they arrive during phase 1 without stealing bandwidth from it.
"""

import sys

import numpy as np

try:
    import concourse  # noqa: F401
except ImportError:  # grading env fallback
    sys.path.insert(0, "/opt/trn_rl_repo")

import ml_dtypes

T, E, D, I = 2048, 8, 1024, 2048
ND = D // 128  # 8 contraction tiles for gate/up
NI = I // 128  # 16 contraction tiles for down

_PROGRAM_CACHE = {}


def _build_program(C):
    """Build + compile the per-core Bass program for token capacity C."""
    import concourse.bass as bass  # noqa: F401
    import concourse.mybir as mybir
    import concourse.tile as tile
    from concourse import bacc

    BF = mybir.dt.bfloat16
    F32 = mybir.dt.float32

    nc = bacc.Bacc(
        "TRN2",
        target_bir_lowering=False,
        debug=False,
        num_devices=E,
        enable_partition_id=False,
    )
    # xT packed: [128, ND*C], partition p / slot d*C+c  <-  x[tok c, d*128+p]
    xT_d = nc.dram_tensor("xT", [128, ND * C], BF, kind="ExternalInput").ap()
    # wg/wu packed I-major: row block i holds [128, D] with
    # [p, d*128+q] <- w[d*128+p, i*128+q]
    wg_d = nc.dram_tensor("wg", [NI * 128, D], BF, kind="ExternalInput").ap()
    wu_d = nc.dram_tensor("wu", [NI * 128, D], BF, kind="ExternalInput").ap()
    # wd natural [I, D] layout; block i = rows i*128..+128
    wd_d = nc.dram_tensor("wd", [I, D], BF, kind="ExternalInput").ap()
    outT_d = nc.dram_tensor("outT", [D, C], F32, kind="ExternalOutput").ap()

    # PSUM bank holds 2KB/partition = 512 fp32: split the moving dim if needed.
    n_chunks = -(-C // 512)
    chunks = [(n * 512, min(512, C - n * 512)) for n in range(n_chunks)]

    with tile.TileContext(nc) as tc:
        with (
            tc.tile_pool(name="xp", bufs=1) as xp,
            tc.tile_pool(name="wp", bufs=1) as wp,
            tc.tile_pool(name="hp", bufs=1) as hp,
            tc.tile_pool(name="sp", bufs=3) as sp,
            tc.tile_pool(name="op", bufs=3) as op,
            tc.tile_pool(name="pg", bufs=2, space="PSUM") as pg,
            tc.tile_pool(name="pu", bufs=2, space="PSUM") as pu,
            tc.tile_pool(name="po", bufs=2, space="PSUM") as po,
        ):
            # x first on the sync ring (small, unblocks the first matmuls).
            xT = xp.tile([128, ND * C], BF, tag="x", name="xT")
            nc.sync.dma_start(xT[:], xT_d[:, :])

            # Phase-1 weights: one 256KB block per (proj, i), gate on the
            # sync HWDGE ring, up on the scalar ring — two queues drain in
            # parallel, and block i is available as soon as it lands.
            wg = []
            wu = []
            for i in range(NI):
                t = wp.tile([128, D], BF, tag=f"wg{i}", name=f"wg{i}")
                nc.sync.dma_start(t[:], wg_d[i * 128 : (i + 1) * 128, :])
                wg.append(t)
                t = wp.tile([128, D], BF, tag=f"wu{i}", name=f"wu{i}")
                nc.scalar.dma_start(t[:], wu_d[i * 128 : (i + 1) * 128, :])
                wu.append(t)
            # Down-projection blocks queue behind on both rings (per-ring
            # FIFO => they only consume bandwidth once phase-1 weights are
            # all in flight).
            wd = []
            for i in range(NI):
                t = wp.tile([128, D], BF, tag=f"wd{i}", name=f"wd{i}")
                eng = nc.sync if i % 2 == 0 else nc.scalar
                eng.dma_start(t[:], wd_d[i * 128 : (i + 1) * 128, :])
                wd.append(t)

            # Phase 1: hidT[i] = silu(Wg^T x^T) * (Wu^T x^T), one 128-row
            # strip of the intermediate dim per iteration.
            hT = []
            for i in range(NI):
                h_t = hp.tile([128, C], BF, tag=f"h{i}", name=f"hT{i}")
                for c0, cn in chunks:
                    csl = bass.ds(c0, cn)
                    g_ps = pg.tile([128, cn], F32, tag="g", name="g_ps")
                    u_ps = pu.tile([128, cn], F32, tag="u", name="u_ps")
                    for d in range(ND):
                        dsl = bass.ds(d * 128, 128)
                        xsl = bass.ds(d * C + c0, cn)
                        nc.tensor.matmul(
                            g_ps[:],
                            wg[i][:, dsl],
                            xT[:, xsl],
                            start=(d == 0),
                            stop=(d == ND - 1),
                        )
                    for d in range(ND):
                        dsl = bass.ds(d * 128, 128)
                        xsl = bass.ds(d * C + c0, cn)
                        nc.tensor.matmul(
                            u_ps[:],
                            wu[i][:, dsl],
                            xT[:, xsl],
                            start=(d == 0),
                            stop=(d == ND - 1),
                        )
                    # silu(g) = g * sigmoid(g); each DVE mul reads at most
                    # one PSUM operand (DVE has a single PSUM read port).
                    s_sb = sp.tile([128, cn], F32, tag="s", name="s_sb")
                    nc.scalar.activation(
                        s_sb[:], g_ps[:], mybir.ActivationFunctionType.Sigmoid
                    )
                    gs_sb = sp.tile([128, cn], F32, tag="gs", name="gs_sb")
                    nc.vector.tensor_mul(gs_sb[:], s_sb[:], g_ps[:])
                    nc.vector.tensor_mul(h_t[:, csl], gs_sb[:], u_ps[:])
                hT.append(h_t)

            # Phase 2: outT[dstrip] = Wd^T @ hidT, accumulated over all 16
            # intermediate strips.
            for d in range(ND):
                dsl = bass.ds(d * 128, 128)
                for c0, cn in chunks:
                    csl = bass.ds(c0, cn)
                    o_ps = po.tile([128, cn], F32, tag="o", name="o_ps")
                    for i in range(NI):
                        nc.tensor.matmul(
                            o_ps[:],
                            wd[i][:, dsl],
                            hT[i][:, csl],
                            start=(i == 0),
                            stop=(i == NI - 1),
                        )
                    o_sb = op.tile([128, cn], F32, tag="ob", name="o_sb")
                    nc.vector.tensor_copy(o_sb[:], o_ps[:])
                    nc.sync.dma_start(outT_d[dsl, csl], o_sb[:])

    nc.compile()
    return nc


def _get_program(C):
    if C not in _PROGRAM_CACHE:
        _PROGRAM_CACHE[C] = _build_program(C)
    return _PROGRAM_CACHE[C]


def _run(nc, in_maps, trace=False):
    from concourse.bass_utils import run_bass_kernel_spmd

    return run_bass_kernel_spmd(nc, in_maps, core_ids=list(range(E)), trace=trace)


def _pack_imajor(w):
    # [D, I] f32 -> [NI*128, D] bf16 with [i*128+p, d*128+q] = w[d*128+p, i*128+q]
    blocks = w.reshape(ND, 128, NI, 128).transpose(2, 1, 0, 3).reshape(NI * 128, D)
    return np.ascontiguousarray(blocks).astype(ml_dtypes.bfloat16)


def kernel(x, expert_indices, w_gate, w_up, w_down, _trace=False, _results=None):
    x = np.asarray(x)
    idx = np.asarray(expert_indices).astype(np.int64)
    counts = np.bincount(idx, minlength=E)
    C = int(max(128, -(-counts.max() // 32) * 32))

    nc = _get_program(C)

    order = np.argsort(idx, kind="stable")
    starts = np.zeros(E + 1, dtype=np.int64)
    np.cumsum(counts, out=starts[1:])

    bf16 = ml_dtypes.bfloat16
    in_maps = []
    for e in range(E):
        toks = order[starts[e] : starts[e + 1]]
        # xT packed: [128, ND*C]; [p, d*C+c] = x[tok c, d*128+p]
        xTg = np.zeros((128, ND, C), dtype=bf16)
        xTg[:, :, : len(toks)] = (
            x[toks].astype(bf16).T.reshape(ND, 128, len(toks)).transpose(1, 0, 2)
        )
        in_maps.append(
            {
                "xT": xTg.reshape(128, ND * C),
                "wg": _pack_imajor(w_gate[e]),
                "wu": _pack_imajor(w_up[e]),
                "wd": np.ascontiguousarray(w_down[e]).astype(bf16),
            }
        )

    res = _run(nc, in_maps, trace=_trace)
    if _results is not None:
        _results.append(res)

    out = np.zeros((T, D), dtype=np.float32)
    for e in range(E):
        toks = order[starts[e] : starts[e + 1]]
        outT = res.results[e]["outT"]  # [D, C] fp32
        out[toks] = outT[:, : len(toks)].T
    return out


# revision 5
# speedup vs baseline: 1.2102x; 1.1977x over previous
"""MoE expert-group kernel for Trainium2 (8 NeuronCores).

Problem: T=2048 tokens, E=8 experts, D=1024, I=2048.
  out[t] = silu(x[t] @ w_gate[e]) * (x[t] @ w_up[e]) @ w_down[e],  e = expert_indices[t]

Strategy: expert parallelism. Host-side (numpy) routing gathers tokens by
expert (this is the "all-to-all"); core e runs expert e's dense
gate/up/silu/down pipeline; host scatters rows back.

On-chip formulation is fully transposed so no transposes are ever needed:
  gateT = Wg^T @ X^T        (stationary = 128x128 Wg block, moving = xT [128, C])
  hidT  = silu(gateT)*upT   (ACT sigmoid + DVE muls, written bf16)
  outT  = Wd^T @ hidT       (stationary = 128x128 Wd block, moving = hT [128, C])

All inputs are cast to bf16 on the host (halves weight DMA, PE runs at full
bf16 rate); accumulation is fp32 in PSUM and the output is fp32.

DMA design (what actually matters on TRN2):
- Each dma_start occupies its issuing engine ~0.6us and rings are FIFO, so
  use few, large (~1-2MB) transfers.
- The Scalar engine must stay DMA-free: its instruction stream also carries
  the sigmoids, and queued DMA triggers would block them (observed 21us
  pipeline stall).
- Two parallel rings: Sync (HWDGE) carries wg + half of wd; GpSimd (SWDGE)
  carries x, wu, the other half of wd. Within a ring, FIFO order makes the
  down-projection stream naturally after the phase-1 weights.
- Host packs weights as [128, I/128 * D] with free index i*D + d*128 + q so
  any block of i-slices is one per-partition-contiguous DMA, and each
  phase-1 step only depends on its own 1MB block.
"""

import sys

import numpy as np

try:
    import concourse  # noqa: F401
except ImportError:  # grading env fallback
    sys.path.insert(0, "/opt/trn_rl_repo")

import ml_dtypes

T, E, D, I = 2048, 8, 1024, 2048
ND = D // 128  # 8 contraction tiles for gate/up
NI = I // 128  # 16 contraction tiles for down
IBLK = 4  # i-slices per wg/wu DMA block (4 * 256KB = 1MB)
NBLK = NI // IBLK
DBLK = NI // 2  # wd ships as 2 blocks of 2MB

_PROGRAM_CACHE = {}


def _build_program(C):
    """Build + compile the per-core Bass program for token capacity C."""
    import concourse.bass as bass  # noqa: F401
    import concourse.mybir as mybir
    import concourse.tile as tile
    from concourse import bacc

    BF = mybir.dt.bfloat16
    F32 = mybir.dt.float32

    nc = bacc.Bacc(
        "TRN2",
        target_bir_lowering=False,
        debug=False,
        num_devices=E,
        enable_partition_id=False,
    )
    # xT packed: [128, ND*C], partition p / slot d*C+c  <-  x[tok c, d*128+p]
    xT_d = nc.dram_tensor("xT", [128, ND * C], BF, kind="ExternalInput").ap()
    # wg/wu/wd packed: [128, NI*D], free slot i*D + d*128 + q  <-
    #   w[d*128+p, i*128+q] for wg/wu (projection [D, I])
    #   w[i*128+p, d*128+q] for wd (projection [I, D])
    wg_d = nc.dram_tensor("wg", [128, NI * D], BF, kind="ExternalInput").ap()
    wu_d = nc.dram_tensor("wu", [128, NI * D], BF, kind="ExternalInput").ap()
    wd_d = nc.dram_tensor("wd", [128, NI * D], BF, kind="ExternalInput").ap()
    outT_d = nc.dram_tensor("outT", [D, C], F32, kind="ExternalOutput").ap()

    # PSUM bank holds 2KB/partition = 512 fp32: split the moving dim if needed.
    n_chunks = -(-C // 512)
    chunks = [(n * 512, min(512, C - n * 512)) for n in range(n_chunks)]

    with tile.TileContext(nc) as tc:
        with (
            tc.tile_pool(name="xp", bufs=1) as xp,
            tc.tile_pool(name="wp", bufs=1) as wp,
            tc.tile_pool(name="hp", bufs=1) as hp,
            tc.tile_pool(name="sp", bufs=3) as sp,
            tc.tile_pool(name="op", bufs=3) as op,
            tc.tile_pool(name="pg", bufs=3, space="PSUM") as pg,
            tc.tile_pool(name="pu", bufs=3, space="PSUM") as pu,
            tc.tile_pool(name="po", bufs=2, space="PSUM") as po,
        ):
            # x on the gpsimd ring, first wg block on the sync ring — both
            # critical-path loads start immediately, in parallel.
            xT = xp.tile([128, ND * C], BF, tag="x", name="xT")
            nc.gpsimd.dma_start(xT[:], xT_d[:, :])

            wg = []
            wu = []
            for b in range(NBLK):
                bsl = bass.ds(b * IBLK * D, IBLK * D)
                t = wp.tile([128, IBLK * D], BF, tag=f"wg{b}", name=f"wg{b}")
                nc.sync.dma_start(t[:], wg_d[:, bsl])
                wg.append(t)
                t = wp.tile([128, IBLK * D], BF, tag=f"wu{b}", name=f"wu{b}")
                nc.gpsimd.dma_start(t[:], wu_d[:, bsl])
                wu.append(t)
            wd = []
            for b in range(2):
                bsl = bass.ds(b * DBLK * D, DBLK * D)
                t = wp.tile([128, DBLK * D], BF, tag=f"wd{b}", name=f"wd{b}")
                eng = nc.sync if b == 0 else nc.gpsimd
                eng.dma_start(t[:], wd_d[:, bsl])
                wd.append(t)

            def wslice(tiles, blk_sz, i, d):
                return tiles[i // blk_sz][:, bass.ds((i % blk_sz) * D + d * 128, 128)]

            # Phase 1: hidT[i] = silu(Wg^T x^T) * (Wu^T x^T), one 128-row
            # strip of the intermediate dim per iteration.
            hT = []
            for i in range(NI):
                h_t = hp.tile([128, C], BF, tag=f"h{i}", name=f"hT{i}")
                for c0, cn in chunks:
                    csl = bass.ds(c0, cn)
                    g_ps = pg.tile([128, cn], F32, tag="g", name="g_ps")
                    u_ps = pu.tile([128, cn], F32, tag="u", name="u_ps")
                    for d in range(ND):
                        xsl = bass.ds(d * C + c0, cn)
                        nc.tensor.matmul(
                            g_ps[:],
                            wslice(wg, IBLK, i, d),
                            xT[:, xsl],
                            start=(d == 0),
                            stop=(d == ND - 1),
                        )
                    for d in range(ND):
                        xsl = bass.ds(d * C + c0, cn)
                        nc.tensor.matmul(
                            u_ps[:],
                            wslice(wu, IBLK, i, d),
                            xT[:, xsl],
                            start=(d == 0),
                            stop=(d == ND - 1),
                        )
                    # silu(g) = g * sigmoid(g); each DVE mul reads at most
                    # one PSUM operand (DVE has a single PSUM read port).
                    s_sb = sp.tile([128, cn], F32, tag="s", name="s_sb")
                    nc.scalar.activation(
                        s_sb[:], g_ps[:], mybir.ActivationFunctionType.Sigmoid
                    )
                    gs_sb = sp.tile([128, cn], F32, tag="gs", name="gs_sb")
                    nc.vector.tensor_mul(gs_sb[:], s_sb[:], g_ps[:])
                    nc.vector.tensor_mul(h_t[:, csl], gs_sb[:], u_ps[:])
                hT.append(h_t)

            # Phase 2: outT[dstrip] = Wd^T @ hidT, accumulated over all 16
            # intermediate strips.
            for d in range(ND):
                dsl = bass.ds(d * 128, 128)
                for c0, cn in chunks:
                    csl = bass.ds(c0, cn)
                    o_ps = po.tile([128, cn], F32, tag="o", name="o_ps")
                    for i in range(NI):
                        nc.tensor.matmul(
                            o_ps[:],
                            wslice(wd, DBLK, i, d),
                            hT[i][:, csl],
                            start=(i == 0),
                            stop=(i == NI - 1),
                        )
                    o_sb = op.tile([128, cn], F32, tag="ob", name="o_sb")
                    nc.vector.tensor_copy(o_sb[:], o_ps[:])
                    nc.sync.dma_start(outT_d[dsl, csl], o_sb[:])

    nc.compile()
    return nc


def _get_program(C):
    if C not in _PROGRAM_CACHE:
        _PROGRAM_CACHE[C] = _build_program(C)
    return _PROGRAM_CACHE[C]


def _run(nc, in_maps, trace=False):
    from concourse.bass_utils import run_bass_kernel_spmd

    return run_bass_kernel_spmd(nc, in_maps, core_ids=list(range(E)), trace=trace)


def _pack_w(w, transpose):
    # -> [128, NI*D] bf16, free slot i*D + d*128 + q
    # transpose=True:  w is [D, I] (wg/wu), block (i,d) = w[d*128:+128, i*128:+128]
    # transpose=False: w is [I, D] (wd),   block (i,d) = w[i*128:+128, d*128:+128]
    if transpose:
        b = w.reshape(ND, 128, NI, 128).transpose(1, 2, 0, 3)  # p, i, d, q
    else:
        b = w.reshape(NI, 128, ND, 128).transpose(1, 0, 2, 3)  # p, i, d, q
    return np.ascontiguousarray(b.reshape(128, NI * D)).astype(ml_dtypes.bfloat16)


def kernel(x, expert_indices, w_gate, w_up, w_down, _trace=False, _results=None):
    x = np.asarray(x)
    idx = np.asarray(expert_indices).astype(np.int64)
    counts = np.bincount(idx, minlength=E)
    C = int(max(128, -(-counts.max() // 32) * 32))

    nc = _get_program(C)

    order = np.argsort(idx, kind="stable")
    starts = np.zeros(E + 1, dtype=np.int64)
    np.cumsum(counts, out=starts[1:])

    bf16 = ml_dtypes.bfloat16
    in_maps = []
    for e in range(E):
        toks = order[starts[e] : starts[e + 1]]
        # xT packed: [128, ND*C]; [p, d*C+c] = x[tok c, d*128+p]
        xTg = np.zeros((128, ND, C), dtype=bf16)
        xTg[:, :, : len(toks)] = (
            x[toks].astype(bf16).T.reshape(ND, 128, len(toks)).transpose(1, 0, 2)
        )
        in_maps.append(
            {
                "xT": xTg.reshape(128, ND * C),
                "wg": _pack_w(w_gate[e], True),
                "wu": _pack_w(w_up[e], True),
                "wd": _pack_w(w_down[e], False),
            }
        )

    res = _run(nc, in_maps, trace=_trace)
    if _results is not None:
        _results.append(res)

    out = np.zeros((T, D), dtype=np.float32)
    for e in range(E):
        toks = order[starts[e] : starts[e + 1]]
        outT = res.results[e]["outT"]  # [D, C] fp32
        out[toks] = outT[:, : len(toks)].T
    return out


# revision 6
# speedup vs baseline: 1.2123x; 1.0017x over previous
"""MoE expert-group kernel for Trainium2 (8 NeuronCores).

Problem: T=2048 tokens, E=8 experts, D=1024, I=2048.
  out[t] = silu(x[t] @ w_gate[e]) * (x[t] @ w_up[e]) @ w_down[e],  e = expert_indices[t]

Strategy: expert parallelism. Host-side (numpy) routing gathers tokens by
expert (this is the "all-to-all"); core e runs expert e's dense
gate/up/silu/down pipeline; host scatters rows back.

On-chip formulation is fully transposed so no transposes are ever needed:
  gateT = Wg^T @ X^T        (stationary = 128x128 Wg block, moving = xT [128, C])
  hidT  = silu(gateT)*upT   (ACT sigmoid + DVE muls, written bf16)
  outT  = Wd^T @ hidT       (stationary = 128x128 Wd block, moving = hT [128, C])

All inputs are cast to bf16 on the host (halves weight DMA, PE runs at full
bf16 rate); accumulation is fp32 in PSUM and the output is fp32.

DMA design (what actually matters on TRN2):
- Each dma_start occupies its issuing engine ~0.6us and rings are FIFO, so
  use few, large (~1-2MB) transfers.
- The Scalar engine must stay DMA-free: its instruction stream also carries
  the sigmoids, and queued DMA triggers would block them (observed 21us
  pipeline stall).
- Two parallel rings: Sync (HWDGE) carries wg + half of wd; GpSimd (SWDGE)
  carries x, wu, the other half of wd. Within a ring, FIFO order makes the
  down-projection stream naturally after the phase-1 weights.
- Host packs weights as [128, I/128 * D] with free index i*D + d*128 + q so
  any block of i-slices is one per-partition-contiguous DMA, and each
  phase-1 step only depends on its own 1MB block.
"""

import sys

import numpy as np

try:
    import concourse  # noqa: F401
except ImportError:  # grading env fallback
    sys.path.insert(0, "/opt/trn_rl_repo")

import ml_dtypes

T, E, D, I = 2048, 8, 1024, 2048
ND = D // 128  # 8 contraction tiles for gate/up
NI = I // 128  # 16 contraction tiles for down
IBLK = 4  # i-slices per wg/wu DMA block (4 * 256KB = 1MB)
NBLK = NI // IBLK
DBLK = NI // 2  # wd ships as 2 blocks of 2MB

_PROGRAM_CACHE = {}


def _build_program(C):
    """Build + compile the per-core Bass program for token capacity C."""
    import concourse.bass as bass  # noqa: F401
    import concourse.mybir as mybir
    import concourse.tile as tile
    from concourse import bacc

    BF = mybir.dt.bfloat16
    F32 = mybir.dt.float32

    nc = bacc.Bacc(
        "TRN2",
        target_bir_lowering=False,
        debug=False,
        num_devices=E,
        enable_partition_id=False,
    )
    # xT packed: [128, ND*C], partition p / slot d*C+c  <-  x[tok c, d*128+p]
    xT_d = nc.dram_tensor("xT", [128, ND * C], BF, kind="ExternalInput").ap()
    # wg/wu/wd packed: [128, NI*D], free slot i*D + d*128 + q  <-
    #   w[d*128+p, i*128+q] for wg/wu (projection [D, I])
    #   w[i*128+p, d*128+q] for wd (projection [I, D])
    wg_d = nc.dram_tensor("wg", [128, NI * D], BF, kind="ExternalInput").ap()
    wu_d = nc.dram_tensor("wu", [128, NI * D], BF, kind="ExternalInput").ap()
    wd_d = nc.dram_tensor("wd", [128, NI * D], BF, kind="ExternalInput").ap()
    outT_d = nc.dram_tensor("outT", [D, C], F32, kind="ExternalOutput").ap()

    # PSUM bank holds 2KB/partition = 512 fp32: split the moving dim if needed.
    n_chunks = -(-C // 512)
    chunks = [(n * 512, min(512, C - n * 512)) for n in range(n_chunks)]

    with tile.TileContext(nc) as tc:
        with (
            tc.tile_pool(name="xp", bufs=1) as xp,
            tc.tile_pool(name="wp", bufs=1) as wp,
            tc.tile_pool(name="hp", bufs=1) as hp,
            tc.tile_pool(name="sp", bufs=3) as sp,
            tc.tile_pool(name="op", bufs=3) as op,
            tc.tile_pool(name="pg", bufs=3, space="PSUM") as pg,
            tc.tile_pool(name="pu", bufs=3, space="PSUM") as pu,
            tc.tile_pool(name="po", bufs=2, space="PSUM") as po,
        ):
            # x first on the sync ring (HWDGE starts ~3us earlier than the
            # SWDGE ring and every phase-1 matmul needs x).
            xT = xp.tile([128, ND * C], BF, tag="x", name="xT")
            nc.sync.dma_start(xT[:], xT_d[:, :])

            wg = []
            wu = []
            for b in range(NBLK):
                bsl = bass.ds(b * IBLK * D, IBLK * D)
                t = wp.tile([128, IBLK * D], BF, tag=f"wg{b}", name=f"wg{b}")
                nc.sync.dma_start(t[:], wg_d[:, bsl])
                wg.append(t)
                t = wp.tile([128, IBLK * D], BF, tag=f"wu{b}", name=f"wu{b}")
                nc.gpsimd.dma_start(t[:], wu_d[:, bsl])
                wu.append(t)
            wd = []
            for b in range(2):
                bsl = bass.ds(b * DBLK * D, DBLK * D)
                t = wp.tile([128, DBLK * D], BF, tag=f"wd{b}", name=f"wd{b}")
                eng = nc.sync if b == 0 else nc.gpsimd
                eng.dma_start(t[:], wd_d[:, bsl])
                wd.append(t)

            def wslice(tiles, blk_sz, i, d):
                return tiles[i // blk_sz][:, bass.ds((i % blk_sz) * D + d * 128, 128)]

            # Phase 1: hidT[i] = silu(Wg^T x^T) * (Wu^T x^T), one 128-row
            # strip of the intermediate dim per iteration.
            hT = []
            for i in range(NI):
                h_t = hp.tile([128, C], BF, tag=f"h{i}", name=f"hT{i}")
                for c0, cn in chunks:
                    csl = bass.ds(c0, cn)
                    g_ps = pg.tile([128, cn], F32, tag="g", name="g_ps")
                    u_ps = pu.tile([128, cn], F32, tag="u", name="u_ps")
                    for d in range(ND):
                        xsl = bass.ds(d * C + c0, cn)
                        nc.tensor.matmul(
                            g_ps[:],
                            wslice(wg, IBLK, i, d),
                            xT[:, xsl],
                            start=(d == 0),
                            stop=(d == ND - 1),
                        )
                    for d in range(ND):
                        xsl = bass.ds(d * C + c0, cn)
                        nc.tensor.matmul(
                            u_ps[:],
                            wslice(wu, IBLK, i, d),
                            xT[:, xsl],
                            start=(d == 0),
                            stop=(d == ND - 1),
                        )
                    # silu(g) = g * sigmoid(g); each DVE mul reads at most
                    # one PSUM operand (DVE has a single PSUM read port).
                    s_sb = sp.tile([128, cn], F32, tag="s", name="s_sb")
                    nc.scalar.activation(
                        s_sb[:], g_ps[:], mybir.ActivationFunctionType.Sigmoid
                    )
                    gs_sb = sp.tile([128, cn], F32, tag="gs", name="gs_sb")
                    nc.vector.tensor_mul(gs_sb[:], s_sb[:], g_ps[:])
                    nc.vector.tensor_mul(h_t[:, csl], gs_sb[:], u_ps[:])
                hT.append(h_t)

            # Phase 2: outT[dstrip] = Wd^T @ hidT, accumulated over all 16
            # intermediate strips.
            for d in range(ND):
                dsl = bass.ds(d * 128, 128)
                for c0, cn in chunks:
                    csl = bass.ds(c0, cn)
                    o_ps = po.tile([128, cn], F32, tag="o", name="o_ps")
                    for i in range(NI):
                        nc.tensor.matmul(
                            o_ps[:],
                            wslice(wd, DBLK, i, d),
                            hT[i][:, csl],
                            start=(i == 0),
                            stop=(i == NI - 1),
                        )
                    o_sb = op.tile([128, cn], F32, tag="ob", name="o_sb")
                    nc.vector.tensor_copy(o_sb[:], o_ps[:])
                    nc.sync.dma_start(outT_d[dsl, csl], o_sb[:])

    nc.compile()
    return nc


def _get_program(C):
    if C not in _PROGRAM_CACHE:
        _PROGRAM_CACHE[C] = _build_program(C)
    return _PROGRAM_CACHE[C]


def _run(nc, in_maps, trace=False):
    from concourse.bass_utils import run_bass_kernel_spmd

    return run_bass_kernel_spmd(nc, in_maps, core_ids=list(range(E)), trace=trace)


def _pack_w(w, transpose):
    # -> [128, NI*D] bf16, free slot i*D + d*128 + q
    # transpose=True:  w is [D, I] (wg/wu), block (i,d) = w[d*128:+128, i*128:+128]
    # transpose=False: w is [I, D] (wd),   block (i,d) = w[i*128:+128, d*128:+128]
    if transpose:
        b = w.reshape(ND, 128, NI, 128).transpose(1, 2, 0, 3)  # p, i, d, q
    else:
        b = w.reshape(NI, 128, ND, 128).transpose(1, 0, 2, 3)  # p, i, d, q
    return np.ascontiguousarray(b.reshape(128, NI * D)).astype(ml_dtypes.bfloat16)


def kernel(x, expert_indices, w_gate, w_up, w_down, _trace=False, _results=None):
    x = np.asarray(x)
    idx = np.asarray(expert_indices).astype(np.int64)
    counts = np.bincount(idx, minlength=E)
    C = int(max(128, -(-counts.max() // 32) * 32))

    nc = _get_program(C)

    order = np.argsort(idx, kind="stable")
    starts = np.zeros(E + 1, dtype=np.int64)
    np.cumsum(counts, out=starts[1:])

    bf16 = ml_dtypes.bfloat16
    in_maps = []
    for e in range(E):
        toks = order[starts[e] : starts[e + 1]]
        # xT packed: [128, ND*C]; [p, d*C+c] = x[tok c, d*128+p]
        xTg = np.zeros((128, ND, C), dtype=bf16)
        xTg[:, :, : len(toks)] = (
            x[toks].astype(bf16).T.reshape(ND, 128, len(toks)).transpose(1, 0, 2)
        )
        in_maps.append(
            {
                "xT": xTg.reshape(128, ND * C),
                "wg": _pack_w(w_gate[e], True),
                "wu": _pack_w(w_up[e], True),
                "wd": _pack_w(w_down[e], False),
            }
        )

    res = _run(nc, in_maps, trace=_trace)
    if _results is not None:
        _results.append(res)

    out = np.zeros((T, D), dtype=np.float32)
    for e in range(E):
        toks = order[starts[e] : starts[e + 1]]
        outT = res.results[e]["outT"]  # [D, C] fp32
        out[toks] = outT[:, : len(toks)].T
    return out
